# revision 16
# baseline (speedup 1.0000x reference)
"""Trainium2 Bass kernel for causal self-attention (GQA + RoPE + QK-rmsnorm).

Problem: B=2, T=2048, C=2048, H=16 q-heads, KVH=4 kv-heads, HD=128.
Sharding: 8 cores = 2 batches (data parallel) x 4 kv-head groups (tensor
parallel). Each core computes its batch's 4 q-heads / 1 kv-head slice plus a
partial output projection; the host sums the 4 partials per batch.

Per-core pipeline (all matmuls in float32r: fp32 storage, ~4x fp32 speed):
  phase 1: Q^T/K^T/V^T = W^T-tiles.T @ x^T  (projection, transposed layout)
           + RoPE + rmsnorm epilogue on Q/K, PE-transpose epilogue on V
  phase 2: S^T = K^T-tile.T @ Q^T, exp on ACT, causal mask, softmax-denominator
           accumulation on DVE, O~^T = V.T @ Pexp^T, normalize via rank-1
           reciprocal broadcast
  phase 3: out += O^T-tile.T @ wo-slice  (partial output projection)
"""

import numpy as np

import bass_rust
import concourse.bass as bass
import concourse.mybir as mybir
import concourse.tile as tile
from concourse import bass_utils

F32 = mybir.dt.float32
F32R = mybir.dt.float32r

B, T, C = 2, 2048, 2048
H, KVH, HD = 16, 4, 128
HL = H // KVH          # q-heads per core (local)
P = 128                # partitions / tile edge
TB = 512               # t/i block (moving free dim)
NTB = T // TB          # 4
NC = C // P            # 16 contraction tiles
SCALE = float(HD) ** -0.25  # folded into both q and k norms
PIPELINE = False


def split_multi_waits(nc):
    """This container's walrus accepts at most ONE sync-wait per instruction;
    Tile can attach several. Move extras onto single-wait NoOps inserted just
    before the instruction on the same engine."""
    seq = 0
    for f in nc.m.functions:
        for bb in f.blocks:
            new = []
            changed = False
            for inst in bb.instructions:
                si = inst.sync_info
                waits = list(si.on_wait) if (si is not None and si.on_wait) else []
                if len(waits) > 1 and type(inst).__name__ != "DMACopy":
                    for w in waits[:-1]:
                        nop = bass_rust.InstNoOp(name=f"I-waitsplit-{seq}")
                        seq += 1
                        nop.engine = inst.engine
                        nop.sync_info = mybir.SyncInfo(on_wait=[w], on_update=[])
                        nc.register_instruction(nop, overwrite=True)
                        new.append(nop)
                    si.on_wait = waits[-1:]
                    changed = True
                new.append(inst)
            if changed:
                bb.instructions = new


def build_nc():
    nc = bass.Bass()

    xt = nc.declare_dram_parameter("xt", [C, T], F32R, isOutput=False)
    wq = nc.declare_dram_parameter("wq", [C, HL * HD], F32R, isOutput=False)
    wk = nc.declare_dram_parameter("wk", [C, HD], F32R, isOutput=False)
    wv = nc.declare_dram_parameter("wv", [C, HD], F32R, isOutput=False)
    wo = nc.declare_dram_parameter("wo", [HL * HD, C], F32R, isOutput=False)
    cc = nc.declare_dram_parameter("cc", [P, T], F32, isOutput=False)
    ssn = nc.declare_dram_parameter("ssn", [P, T], F32, isOutput=False)
    masks = nc.declare_dram_parameter("masks", [P, HL * TB], F32R, isOutput=False)
    qkw_row = nc.declare_dram_parameter("qkw_row", [1, P], F32R, isOutput=False)
    cols = nc.declare_dram_parameter("cols", [P, 2], F32R, isOutput=False)
    onr = nc.declare_dram_parameter("onr", [1, P], F32R, isOutput=False)
    ident = nc.declare_dram_parameter("ident", [P, P], F32R, isOutput=False)
    out = nc.declare_dram_parameter("out", [T, C], F32, isOutput=True)

    Exp = mybir.ActivationFunctionType.Exp
    Ln = mybir.ActivationFunctionType.Ln
    Copy = mybir.ActivationFunctionType.Copy

    with tile.TileContext(nc) as tc:
        with (
            tc.tile_pool(name="const", bufs=1) as const,
            tc.tile_pool(name="wpool", bufs=1) as wpool,
            tc.tile_pool(name="kv", bufs=1) as kvpool,
            tc.tile_pool(name="qt", bufs=2) as qtpool,
            tc.tile_pool(name="ot", bufs=2) as otpool,
            tc.tile_pool(name="xtp", bufs=3) as xtp,
            tc.tile_pool(name="work", bufs=2) as work,
            tc.tile_pool(name="pexp", bufs=3) as pexpp,
            tc.tile_pool(name="osb", bufs=2) as osb,
            tc.tile_pool(name="small", bufs=1) as small,
            tc.tile_pool(name="ps", bufs=1, space="PSUM") as ps,
        ):
            # ---- resident constants / weights ----
            masks_sb = const.tile([P, HL, TB], F32R, tag="masks")
            nc.sync.dma_start(out=masks_sb, in_=masks[:, :].rearrange("p (r i) -> p r i", r=HL))
            qkw_sb = const.tile([1, P], F32R, tag="qkw")
            nc.sync.dma_start(out=qkw_sb, in_=qkw_row[:, :])
            cols_sb = const.tile([P, 2], F32R, tag="cols")
            nc.sync.dma_start(out=cols_sb, in_=cols[:, :])
            onr_sb = const.tile([1, P], F32R, tag="onr")
            nc.sync.dma_start(out=onr_sb, in_=onr[:, :])
            id_sb = const.tile([P, P], F32R, tag="ident")
            nc.sync.dma_start(out=id_sb, in_=ident[:, :])
            eps_sb = const.tile([1, 1], F32, tag="eps")
            nc.vector.memset(eps_sb, 1e-6)
            cc_sb = const.tile([P, T], F32, tag="cc")
            nc.sync.dma_start(out=cc_sb, in_=cc[:, :])
            ssn_sb = const.tile([P, T], F32, tag="ssn")
            nc.sync.dma_start(out=ssn_sb, in_=ssn[:, :])

            wq_sb = wpool.tile([P, NC, HL * HD], F32R, tag="wq")
            nc.sync.dma_start(out=wq_sb, in_=wq[:, :].rearrange("(c p) f -> p c f", p=P))
            wk_sb = wpool.tile([P, NC, HD], F32R, tag="wk")
            nc.sync.dma_start(out=wk_sb, in_=wk[:, :].rearrange("(c p) f -> p c f", p=P))
            wv_sb = wpool.tile([P, NC, HD], F32R, tag="wv")
            nc.sync.dma_start(out=wv_sb, in_=wv[:, :].rearrange("(c p) f -> p c f", p=P))
            wo_sb = wpool.tile([P, HL, C], F32R, tag="wo")
            nc.sync.dma_start(out=wo_sb, in_=wo[:, :].rearrange("(h p) f -> p h f", p=P))

            # K^T and V-natural stay resident across the whole kernel
            kt_sb = kvpool.tile([P, T], F32R, tag="kt")
            vn_sb = kvpool.tile([P, NC, HD], F32R, tag="vn")

            def rope_rms_epilogue(src_ps, dst, cc_t, ssn_t):
                """src_ps: [128(d), 512(t)] PSUM projection block (pre-norm Q^T/K^T).
                dst: [128, 512] F32R SBUF destination slice."""
                # PSUM -> SBUF once on ACT so the DVE ops below run in 2x mode
                s_sb = work.tile([P, TB], F32, tag="src")
                nc.scalar.activation(out=s_sb, in_=src_ps, func=Copy)
                t1 = work.tile([P, TB], F32, tag="t1")
                nc.vector.tensor_mul(t1, s_sb, cc_t)
                t2 = work.tile([P, TB], F32, tag="t2")
                # ssn is host-arranged as [+sin; -sin] so each half-mul's two
                # SBUF inputs share a base partition (HW constraint)
                nc.vector.tensor_mul(t2[0:64, :], s_sb[64:128, :], ssn_t[64:128, :])
                nc.vector.tensor_mul(t2[64:128, :], s_sb[0:64, :], ssn_t[0:64, :])
                rot = t1
                nc.vector.tensor_add(rot, t1, t2)
                sq = work.tile([P, TB], F32R, tag="t2")
                nc.vector.tensor_mul(sq, rot, rot)
                var_ps = ps.tile([1, TB], F32, tag="aux", name="var")
                nc.tensor.matmul(var_ps, cols_sb[:, 1:2], sq, start=True, stop=True)
                # 1/sqrt(var+eps) = exp(-0.5 * ln(var+eps)); Ln/Exp share the
                # ACT function table with Exp/Copy so no table switches
                lnv = small.tile([1, TB], F32, tag="lnv")
                nc.scalar.activation(out=lnv, in_=var_ps, func=Ln, bias=eps_sb, scale=1.0)
                rstd_inv = small.tile([1, TB], F32R, tag="rstdi")
                nc.scalar.activation(out=rstd_inv, in_=lnv, func=Exp, scale=-0.5)
                bc_ps = ps.tile([P, TB], F32, tag="aux", name="bc")
                nc.tensor.matmul(bc_ps, qkw_sb, rstd_inv, start=True, stop=True)
                nc.vector.tensor_mul(dst, rot, bc_ps)

            qt_tiles = {}
            ot_tiles = {}

            def emit_phase1(tb):
                tsl = bass.ts(tb, TB)
                cc_t = cc_sb[:, tsl]
                ssn_t = ssn_sb[:, tsl]
                qt_sb = qtpool.tile([P, HL, TB], F32R, tag="qt", name="qt")
                qt_tiles[tb] = qt_sb
                rounds = [
                    [("q", 0), ("q", 1)],
                    [("q", 2), ("q", 3)],
                    [("k", 0), ("v", 0)],
                ]
                for rnd in rounds:
                    pts = {}
                    for idx, (kind, h) in enumerate(rnd):
                        pts[(kind, h)] = ps.tile([P, TB], F32, tag=f"proj{idx}", name=f"proj_{kind}{h}")
                    for c in range(NC):
                        xt_t = xtp.tile([P, TB], F32R, tag="xt", name="xt")
                        nc.sync.dma_start(out=xt_t, in_=xt[c * P:(c + 1) * P, tsl])
                        for kind, h in rnd:
                            if kind == "q":
                                lhsT = wq_sb[:, c, h * HD:(h + 1) * HD]
                            elif kind == "k":
                                lhsT = wk_sb[:, c, :]
                            else:
                                lhsT = wv_sb[:, c, :]
                            nc.tensor.matmul(
                                pts[(kind, h)], lhsT, xt_t,
                                start=(c == 0), stop=(c == NC - 1),
                            )
                    for kind, h in rnd:
                        src = pts[(kind, h)]
                        if kind == "q":
                            rope_rms_epilogue(src, qt_sb[:, h, :], cc_t, ssn_t)
                        elif kind == "k":
                            rope_rms_epilogue(src, kt_sb[:, tsl], cc_t, ssn_t)
                        else:
                            vt_sb = work.tile([P, TB], F32R, tag="vt", name="vt")
                            nc.vector.tensor_copy(out=vt_sb, in_=src)
                            for k in range(TB // P):
                                tr_ps = ps.tile([P, P], F32R, tag="aux", name="vtr")
                                nc.tensor.transpose(tr_ps, vt_sb[:, k * P:(k + 1) * P], id_sb)
                                nc.vector.tensor_copy(out=vn_sb[:, NTB * tb + k, :], in_=tr_ps)

            def emit_phase2(tb):
                nj = (HL * tb) + HL  # causal: j-tiles 0 .. 4*tb+3
                qt_sb = qt_tiles[tb]
                ot_sb = otpool.tile([P, HL, TB], F32R, tag="ot", name="ot")
                ot_tiles[tb] = ot_sb
                for h in range(HL):
                    o_ps = ps.tile([P, TB], F32, tag="oacc", name="oacc", bufs=2)
                    lacc = work.tile([P, TB], F32R, tag="lacc", bufs=2, name="lacc")
                    for j in range(nj):
                        sc_ps = ps.tile([P, TB], F32, tag="sc", name="sc", bufs=3)
                        nc.tensor.matmul(
                            sc_ps, kt_sb[:, j * P:(j + 1) * P], qt_sb[:, h, :],
                            start=True, stop=True,
                        )
                        pexp = pexpp.tile([P, TB], F32R, tag="pexp", name="pexp")
                        nc.scalar.activation(out=pexp, in_=sc_ps, func=Exp)
                        r = j - HL * tb
                        if r >= 0:
                            nc.vector.tensor_mul(pexp, pexp, masks_sb[:, r, :])
                        if j == 0:
                            nc.vector.tensor_copy(out=lacc, in_=pexp)
                        else:
                            nc.vector.tensor_add(lacc, lacc, pexp)
                        nc.tensor.matmul(
                            o_ps, vn_sb[:, j, :], pexp,
                            start=(j == 0), stop=(j == nj - 1),
                        )
                    l_ps = ps.tile([1, TB], F32, tag="aux", name="lsum")
                    nc.tensor.matmul(l_ps, cols_sb[:, 0:1], lacc, start=True, stop=True)
                    lnl = small.tile([1, TB], F32, tag="lnl", name="lnl")
                    nc.scalar.activation(out=lnl, in_=l_ps, func=Ln)
                    linv = small.tile([1, TB], F32R, tag="linv", name="linv")
                    nc.scalar.activation(out=linv, in_=lnl, func=Exp, scale=-1.0)
                    bc2_ps = ps.tile([P, TB], F32, tag="aux", name="lbc")
                    nc.tensor.matmul(bc2_ps, onr_sb, linv, start=True, stop=True)
                    bc2_sb = work.tile([P, TB], F32, tag="bc2", bufs=1, name="bc2")
                    nc.vector.tensor_copy(out=bc2_sb, in_=bc2_ps)
                    nc.vector.tensor_mul(ot_sb[:, h, :], o_ps, bc2_sb)

            def emit_phase3(tb):
                ot_sb = ot_tiles[tb]
                for ts_i in range(TB // P):
                    trow = tb * TB + ts_i * P
                    for nb in range(C // TB):
                        out_ps = ps.tile([P, TB], F32, tag="aux", name="outp")
                        for h in range(HL):
                            nc.tensor.matmul(
                                out_ps,
                                ot_sb[:, h, ts_i * P:(ts_i + 1) * P],
                                wo_sb[:, h, nb * TB:(nb + 1) * TB],
                                start=(h == 0), stop=(h == HL - 1),
                            )
                        o_out = osb.tile([P, TB], F32, tag="osb", name="osb")
                        if (ts_i + nb) % 2 == 0:
                            nc.vector.tensor_copy(out=o_out, in_=out_ps)
                        else:
                            nc.scalar.activation(out=o_out, in_=out_ps, func=Copy)
                        nc.sync.dma_start(
                            out=out[trow:trow + P, nb * TB:(nb + 1) * TB], in_=o_out
                        )

            if PIPELINE:
                emit_phase1(0)
                emit_phase1(1)
                for tb in range(NTB):
                    emit_phase2(tb)
                    emit_phase3(tb)
                    if tb + 2 < NTB:
                        emit_phase1(tb + 2)
            else:
                for tb in range(NTB):
                    emit_phase1(tb)
                    emit_phase2(tb)
                    emit_phase3(tb)

    split_multi_waits(nc)
    return nc


_CACHED = {}


def _get_nc():
    if "nc" not in _CACHED:
        _CACHED["nc"] = build_nc()
    return _CACHED["nc"]


def _host_inputs(x, cos, sin, wq, wk, wv, wo, qk_w):
    """Build the 8 per-core input maps."""
    x = np.asarray(x, np.float32)
    cos = np.asarray(cos, np.float32)
    sin = np.asarray(sin, np.float32)
    wq = np.asarray(wq, np.float32)
    wk = np.asarray(wk, np.float32)
    wv = np.asarray(wv, np.float32)
    wo = np.asarray(wo, np.float32)
    qk_w = np.asarray(qk_w, np.float32)

    cc = np.ascontiguousarray(np.concatenate([cos.T, cos.T], axis=0))      # [128, T]
    # [ +sin ; -sin ]: row d<64 holds sin (pairs with q[d] into rot[d+64]),
    # row d>=64 holds -sin (pairs with q[d] into rot[d-64])
    ssn = np.ascontiguousarray(np.concatenate([sin.T, -sin.T], axis=0))    # [128, T]
    jl = np.arange(P)[:, None]
    il = np.arange(TB)[None, :]
    masks = np.concatenate(
        [(jl + P * r <= il).astype(np.float32) for r in range(HL)], axis=1
    )                                                                      # [128, 4*512]
    qkw_row = np.ascontiguousarray((qk_w * SCALE)[None, :])                # [1, 128]
    cols = np.stack(
        [np.ones(P, np.float32), np.full(P, 1.0 / HD, np.float32)], axis=1
    )                                                                      # [128, 2]
    onr = np.ones((1, P), np.float32)
    ident = np.eye(P, dtype=np.float32)

    ins = []
    for core in range(8):
        b, g = divmod(core, KVH)
        ins.append({
            "xt": np.ascontiguousarray(x[b].T),
            "wq": np.ascontiguousarray(wq[:, g * HL * HD:(g + 1) * HL * HD]),
            "wk": np.ascontiguousarray(wk[:, g * HD:(g + 1) * HD]),
            "wv": np.ascontiguousarray(wv[:, g * HD:(g + 1) * HD]),
            "wo": np.ascontiguousarray(wo[g * HL * HD:(g + 1) * HL * HD, :]),
            "cc": cc, "ssn": ssn, "masks": masks, "qkw_row": qkw_row,
            "cols": cols, "onr": onr, "ident": ident,
        })
    return ins


def run_cores(ins, trace=False, **kwargs):
    nc = _get_nc()
    return bass_utils.run_bass_kernel_spmd(
        nc, ins, list(range(8)), trace=trace, **kwargs
    )


def kernel(x, cos, sin, wq, wk, wv, wo, qk_w):
    ins = _host_inputs(x, cos, sin, wq, wk, wv, wo, qk_w)
    res = run_cores(ins, trace=False)
    out = np.zeros((B, T, C), np.float32)
    for core in range(8):
        b = core // KVH
        out[b] += res.results[core]["out"]
    return out


# revision 17
# speedup vs baseline: 1.1105x; 1.1105x over previous
"""Trainium2 Bass kernel for causal self-attention (GQA + RoPE + QK-rmsnorm).

Problem: B=2, T=2048, C=2048, H=16 q-heads, KVH=4 kv-heads, HD=128.
Sharding: 8 cores = 2 batches (data parallel) x 4 kv-head groups (tensor
parallel). Each core computes its batch's 4 q-heads / 1 kv-head slice plus a
partial output projection; the host sums the 4 partials per batch.

Per-core pipeline (all matmuls in float32r: fp32 storage, ~4x fp32 speed):
  phase 1: Q^T/K^T/V^T = W^T-tiles.T @ x^T  (projection, transposed layout)
           + RoPE + rmsnorm epilogue on Q/K, PE-transpose epilogue on V
  phase 2: S^T = K^T-tile.T @ Q^T, exp on ACT, causal mask, softmax-denominator
           accumulation on DVE, O~^T = V.T @ Pexp^T, normalize via rank-1
           reciprocal broadcast
  phase 3: out += O^T-tile.T @ wo-slice  (partial output projection)
"""

import numpy as np

import bass_rust
import concourse.bass as bass
import concourse.mybir as mybir
import concourse.tile as tile
from concourse import bass_utils

F32 = mybir.dt.float32
F32R = mybir.dt.float32r

B, T, C = 2, 2048, 2048
H, KVH, HD = 16, 4, 128
HL = H // KVH          # q-heads per core (local)
P = 128                # partitions / tile edge
TB = 512               # t/i block (moving free dim)
NTB = T // TB          # 4
NC = C // P            # 16 contraction tiles
SCALE = float(HD) ** -0.25  # folded into both q and k norms
PIPELINE = False


def split_multi_waits(nc):
    """This container's walrus accepts at most ONE sync-wait per instruction;
    Tile can attach several. Move extras onto single-wait NoOps inserted just
    before the instruction on the same engine."""
    seq = 0
    for f in nc.m.functions:
        for bb in f.blocks:
            new = []
            changed = False
            for inst in bb.instructions:
                si = inst.sync_info
                waits = list(si.on_wait) if (si is not None and si.on_wait) else []
                if len(waits) > 1 and type(inst).__name__ != "DMACopy":
                    for w in waits[:-1]:
                        nop = bass_rust.InstNoOp(name=f"I-waitsplit-{seq}")
                        seq += 1
                        nop.engine = inst.engine
                        nop.sync_info = mybir.SyncInfo(on_wait=[w], on_update=[])
                        nc.register_instruction(nop, overwrite=True)
                        new.append(nop)
                    si.on_wait = waits[-1:]
                    changed = True
                new.append(inst)
            if changed:
                bb.instructions = new


def build_nc():
    nc = bass.Bass()

    xt = nc.declare_dram_parameter("xt", [C, T], F32R, isOutput=False)
    wq = nc.declare_dram_parameter("wq", [C, HL * HD], F32R, isOutput=False)
    wk = nc.declare_dram_parameter("wk", [C, HD], F32R, isOutput=False)
    wv = nc.declare_dram_parameter("wv", [C, HD], F32R, isOutput=False)
    wo = nc.declare_dram_parameter("wo", [HL * HD, C], F32R, isOutput=False)
    cc = nc.declare_dram_parameter("cc", [P, T], F32, isOutput=False)
    ssn = nc.declare_dram_parameter("ssn", [P, T], F32, isOutput=False)
    masks = nc.declare_dram_parameter("masks", [P, HL * TB], F32R, isOutput=False)
    qkw_row = nc.declare_dram_parameter("qkw_row", [1, P], F32R, isOutput=False)
    cols = nc.declare_dram_parameter("cols", [P, 2], F32R, isOutput=False)
    onr = nc.declare_dram_parameter("onr", [1, P], F32R, isOutput=False)
    ident = nc.declare_dram_parameter("ident", [P, P], F32R, isOutput=False)
    out = nc.declare_dram_parameter("out", [T, C], F32, isOutput=True)

    Exp = mybir.ActivationFunctionType.Exp
    Ln = mybir.ActivationFunctionType.Ln
    Copy = mybir.ActivationFunctionType.Copy

    with tile.TileContext(nc) as tc:
        with (
            tc.tile_pool(name="const", bufs=1) as const,
            tc.tile_pool(name="wpool", bufs=1) as wpool,
            tc.tile_pool(name="kv", bufs=1) as kvpool,
            tc.tile_pool(name="qt", bufs=2) as qtpool,
            tc.tile_pool(name="ot", bufs=2) as otpool,
            tc.tile_pool(name="xtp", bufs=3) as xtp,
            tc.tile_pool(name="work", bufs=2) as work,
            tc.tile_pool(name="pexp", bufs=3) as pexpp,
            tc.tile_pool(name="osb", bufs=2) as osb,
            tc.tile_pool(name="small", bufs=1) as small,
            tc.tile_pool(name="ps", bufs=1, space="PSUM") as ps,
        ):
            # ---- resident constants / weights ----
            masks_sb = const.tile([P, HL, TB], F32R, tag="masks")
            nc.sync.dma_start(out=masks_sb, in_=masks[:, :].rearrange("p (r i) -> p r i", r=HL))
            qkw_sb = const.tile([1, P], F32R, tag="qkw")
            nc.sync.dma_start(out=qkw_sb, in_=qkw_row[:, :])
            cols_sb = const.tile([P, 2], F32R, tag="cols")
            nc.sync.dma_start(out=cols_sb, in_=cols[:, :])
            onr_sb = const.tile([1, P], F32R, tag="onr")
            nc.sync.dma_start(out=onr_sb, in_=onr[:, :])
            id_sb = const.tile([P, P], F32R, tag="ident")
            nc.sync.dma_start(out=id_sb, in_=ident[:, :])
            eps_sb = const.tile([1, 1], F32, tag="eps")
            nc.vector.memset(eps_sb, 1e-6)
            cc_sb = const.tile([P, T], F32, tag="cc")
            nc.sync.dma_start(out=cc_sb, in_=cc[:, :])
            ssn_sb = const.tile([P, T], F32, tag="ssn")
            nc.sync.dma_start(out=ssn_sb, in_=ssn[:, :])

            wq_sb = wpool.tile([P, NC, HL * HD], F32R, tag="wq")
            nc.sync.dma_start(out=wq_sb, in_=wq[:, :].rearrange("(c p) f -> p c f", p=P))
            wk_sb = wpool.tile([P, NC, HD], F32R, tag="wk")
            nc.sync.dma_start(out=wk_sb, in_=wk[:, :].rearrange("(c p) f -> p c f", p=P))
            wv_sb = wpool.tile([P, NC, HD], F32R, tag="wv")
            nc.sync.dma_start(out=wv_sb, in_=wv[:, :].rearrange("(c p) f -> p c f", p=P))
            wo_sb = wpool.tile([P, HL, C], F32R, tag="wo")
            nc.sync.dma_start(out=wo_sb, in_=wo[:, :].rearrange("(h p) f -> p h f", p=P))

            # K^T and V-natural stay resident across the whole kernel
            kt_sb = kvpool.tile([P, T], F32R, tag="kt")
            vn_sb = kvpool.tile([P, NC, HD], F32R, tag="vn")

            def rope_rms_epilogue(src_ps, dst, cc_t, ssn_t):
                """src_ps: [128(d), 512(t)] PSUM projection block (pre-norm Q^T/K^T).
                dst: [128, 512] F32R SBUF destination slice."""
                # PSUM -> SBUF once on ACT so the DVE ops below run in 2x mode
                s_sb = work.tile([P, TB], F32, tag="src")
                nc.scalar.activation(out=s_sb, in_=src_ps, func=Copy)
                t1 = work.tile([P, TB], F32, tag="t1")
                nc.vector.tensor_mul(t1, s_sb, cc_t)
                t2 = work.tile([P, TB], F32, tag="t2")
                # ssn is host-arranged as [+sin; -sin] so each half-mul's two
                # SBUF inputs share a base partition (HW constraint)
                nc.vector.tensor_mul(t2[0:64, :], s_sb[64:128, :], ssn_t[64:128, :])
                nc.vector.tensor_mul(t2[64:128, :], s_sb[0:64, :], ssn_t[0:64, :])
                rot = t1
                nc.vector.tensor_add(rot, t1, t2)
                sq = work.tile([P, TB], F32R, tag="t2")
                nc.vector.tensor_mul(sq, rot, rot)
                var_ps = ps.tile([1, TB], F32, tag="aux", name="var")
                nc.tensor.matmul(var_ps, cols_sb[:, 1:2], sq, start=True, stop=True)
                # 1/sqrt(var+eps) = exp(-0.5 * ln(var+eps)); Ln/Exp share the
                # ACT function table with Exp/Copy so no table switches
                lnv = small.tile([1, TB], F32, tag="lnv")
                nc.scalar.activation(out=lnv, in_=var_ps, func=Ln, bias=eps_sb, scale=1.0)
                rstd_inv = small.tile([1, TB], F32R, tag="rstdi")
                nc.scalar.activation(out=rstd_inv, in_=lnv, func=Exp, scale=-0.5)
                bc_ps = ps.tile([P, TB], F32, tag="aux", name="bc")
                nc.tensor.matmul(bc_ps, qkw_sb, rstd_inv, start=True, stop=True)
                nc.vector.tensor_mul(dst, rot, bc_ps)

            qt_tiles = {}
            ot_tiles = {}

            def emit_phase1(tb):
                tsl = bass.ts(tb, TB)
                cc_t = cc_sb[:, tsl]
                ssn_t = ssn_sb[:, tsl]
                qt_sb = qtpool.tile([P, HL, TB], F32R, tag="qt", name="qt")
                qt_tiles[tb] = qt_sb
                rounds = [
                    [("q", 0), ("q", 1)],
                    [("q", 2), ("q", 3)],
                    [("k", 0), ("v", 0)],
                ]
                for rnd in rounds:
                    pts = {}
                    for idx, (kind, h) in enumerate(rnd):
                        pts[(kind, h)] = ps.tile([P, TB], F32, tag=f"proj{idx}", name=f"proj_{kind}{h}")
                    for c in range(NC):
                        xt_t = xtp.tile([P, TB], F32R, tag="xt", name="xt")
                        nc.sync.dma_start(out=xt_t, in_=xt[c * P:(c + 1) * P, tsl])
                        for kind, h in rnd:
                            if kind == "q":
                                lhsT = wq_sb[:, c, h * HD:(h + 1) * HD]
                            elif kind == "k":
                                lhsT = wk_sb[:, c, :]
                            else:
                                lhsT = wv_sb[:, c, :]
                            nc.tensor.matmul(
                                pts[(kind, h)], lhsT, xt_t,
                                start=(c == 0), stop=(c == NC - 1),
                            )
                    for kind, h in rnd:
                        src = pts[(kind, h)]
                        if kind == "q":
                            rope_rms_epilogue(src, qt_sb[:, h, :], cc_t, ssn_t)
                        elif kind == "k":
                            rope_rms_epilogue(src, kt_sb[:, tsl], cc_t, ssn_t)
                        else:
                            vt_sb = work.tile([P, TB], F32R, tag="vt", name="vt")
                            nc.vector.tensor_copy(out=vt_sb, in_=src)
                            for k in range(TB // P):
                                tr_ps = ps.tile([P, P], F32R, tag="aux", name="vtr")
                                nc.tensor.transpose(tr_ps, vt_sb[:, k * P:(k + 1) * P], id_sb)
                                nc.vector.tensor_copy(out=vn_sb[:, NTB * tb + k, :], in_=tr_ps)

            def emit_phase2(tb):
                nj = (HL * tb) + HL  # causal: j-tiles 0 .. 4*tb+3
                qt_sb = qt_tiles[tb]
                ot_sb = otpool.tile([P, HL, TB], F32R, tag="ot", name="ot")
                ot_tiles[tb] = ot_sb
                for h in range(HL):
                    o_ps = ps.tile([P, TB], F32, tag="oacc", name="oacc", bufs=1)
                    l_ps = ps.tile([1, TB], F32, tag="lps", name="lsum")
                    for j in range(nj):
                        sc_ps = ps.tile([P, TB], F32, tag="sc", name="sc", bufs=3)
                        nc.tensor.matmul(
                            sc_ps, kt_sb[:, j * P:(j + 1) * P], qt_sb[:, h, :],
                            start=True, stop=True,
                        )
                        pexp = pexpp.tile([P, TB], F32R, tag="pexp", name="pexp", bufs=4)
                        nc.scalar.activation(out=pexp, in_=sc_ps, func=Exp)
                        r = j - HL * tb
                        if r >= 0:
                            nc.vector.tensor_mul(pexp, pexp, masks_sb[:, r, :])
                        nc.tensor.matmul(
                            l_ps, cols_sb[:, 0:1], pexp,
                            start=(j == 0), stop=(j == nj - 1),
                        )
                        nc.tensor.matmul(
                            o_ps, vn_sb[:, j, :], pexp,
                            start=(j == 0), stop=(j == nj - 1),
                        )
                    lnl = small.tile([1, TB], F32, tag="lnl", name="lnl")
                    nc.scalar.activation(out=lnl, in_=l_ps, func=Ln)
                    linv = small.tile([1, TB], F32R, tag="linv", name="linv")
                    nc.scalar.activation(out=linv, in_=lnl, func=Exp, scale=-1.0)
                    bc2_ps = ps.tile([P, TB], F32, tag="aux", name="lbc")
                    nc.tensor.matmul(bc2_ps, onr_sb, linv, start=True, stop=True)
                    bc2_sb = work.tile([P, TB], F32, tag="bc2", bufs=1, name="bc2")
                    nc.vector.tensor_copy(out=bc2_sb, in_=bc2_ps)
                    nc.vector.tensor_mul(ot_sb[:, h, :], o_ps, bc2_sb)

            def emit_phase3(tb):
                ot_sb = ot_tiles[tb]
                for ts_i in range(TB // P):
                    trow = tb * TB + ts_i * P
                    for nb in range(C // TB):
                        out_ps = ps.tile([P, TB], F32, tag="aux", name="outp")
                        for h in range(HL):
                            nc.tensor.matmul(
                                out_ps,
                                ot_sb[:, h, ts_i * P:(ts_i + 1) * P],
                                wo_sb[:, h, nb * TB:(nb + 1) * TB],
                                start=(h == 0), stop=(h == HL - 1),
                            )
                        o_out = osb.tile([P, TB], F32, tag="osb", name="osb")
                        if (ts_i + nb) % 2 == 0:
                            nc.vector.tensor_copy(out=o_out, in_=out_ps)
                        else:
                            nc.scalar.activation(out=o_out, in_=out_ps, func=Copy)
                        nc.sync.dma_start(
                            out=out[trow:trow + P, nb * TB:(nb + 1) * TB], in_=o_out
                        )

            if PIPELINE:
                emit_phase1(0)
                emit_phase1(1)
                for tb in range(NTB):
                    emit_phase2(tb)
                    emit_phase3(tb)
                    if tb + 2 < NTB:
                        emit_phase1(tb + 2)
            else:
                for tb in range(NTB):
                    emit_phase1(tb)
                    emit_phase2(tb)
                    emit_phase3(tb)

    split_multi_waits(nc)
    return nc


_CACHED = {}


def _get_nc():
    if "nc" not in _CACHED:
        _CACHED["nc"] = build_nc()
    return _CACHED["nc"]


def _host_inputs(x, cos, sin, wq, wk, wv, wo, qk_w):
    """Build the 8 per-core input maps."""
    x = np.asarray(x, np.float32)
    cos = np.asarray(cos, np.float32)
    sin = np.asarray(sin, np.float32)
    wq = np.asarray(wq, np.float32)
    wk = np.asarray(wk, np.float32)
    wv = np.asarray(wv, np.float32)
    wo = np.asarray(wo, np.float32)
    qk_w = np.asarray(qk_w, np.float32)

    cc = np.ascontiguousarray(np.concatenate([cos.T, cos.T], axis=0))      # [128, T]
    # [ +sin ; -sin ]: row d<64 holds sin (pairs with q[d] into rot[d+64]),
    # row d>=64 holds -sin (pairs with q[d] into rot[d-64])
    ssn = np.ascontiguousarray(np.concatenate([sin.T, -sin.T], axis=0))    # [128, T]
    jl = np.arange(P)[:, None]
    il = np.arange(TB)[None, :]
    masks = np.concatenate(
        [(jl + P * r <= il).astype(np.float32) for r in range(HL)], axis=1
    )                                                                      # [128, 4*512]
    qkw_row = np.ascontiguousarray((qk_w * SCALE)[None, :])                # [1, 128]
    cols = np.stack(
        [np.ones(P, np.float32), np.full(P, 1.0 / HD, np.float32)], axis=1
    )                                                                      # [128, 2]
    onr = np.ones((1, P), np.float32)
    ident = np.eye(P, dtype=np.float32)

    ins = []
    for core in range(8):
        b, g = divmod(core, KVH)
        ins.append({
            "xt": np.ascontiguousarray(x[b].T),
            "wq": np.ascontiguousarray(wq[:, g * HL * HD:(g + 1) * HL * HD]),
            "wk": np.ascontiguousarray(wk[:, g * HD:(g + 1) * HD]),
            "wv": np.ascontiguousarray(wv[:, g * HD:(g + 1) * HD]),
            "wo": np.ascontiguousarray(wo[g * HL * HD:(g + 1) * HL * HD, :]),
            "cc": cc, "ssn": ssn, "masks": masks, "qkw_row": qkw_row,
            "cols": cols, "onr": onr, "ident": ident,
        })
    return ins


def run_cores(ins, trace=False, **kwargs):
    nc = _get_nc()
    return bass_utils.run_bass_kernel_spmd(
        nc, ins, list(range(8)), trace=trace, **kwargs
    )


def kernel(x, cos, sin, wq, wk, wv, wo, qk_w):
    ins = _host_inputs(x, cos, sin, wq, wk, wv, wo, qk_w)
    res = run_cores(ins, trace=False)
    out = np.zeros((B, T, C), np.float32)
    for core in range(8):
        b = core // KVH
        out[b] += res.results[core]["out"]
    return out


# revision 18
# speedup vs baseline: 1.3325x; 1.2000x over previous
"""Trainium2 Bass kernel for causal self-attention (GQA + RoPE + QK-rmsnorm).

Problem: B=2, T=2048, C=2048, H=16 q-heads, KVH=4 kv-heads, HD=128.
Sharding: 8 cores = 2 batches (data parallel) x 4 kv-head groups (tensor
parallel). Each core computes its batch's 4 q-heads / 1 kv-head slice plus a
partial output projection; the host sums the 4 partials per batch.

Per-core pipeline (all matmuls in float32r: fp32 storage, ~4x fp32 speed):
  phase 1: Q^T/K^T/V^T = W^T-tiles.T @ x^T  (projection, transposed layout)
           + RoPE + rmsnorm epilogue on Q/K, PE-transpose epilogue on V
  phase 2: S^T = K^T-tile.T @ Q^T, exp on ACT, causal mask, softmax-denominator
           accumulation on DVE, O~^T = V.T @ Pexp^T, normalize via rank-1
           reciprocal broadcast
  phase 3: out += O^T-tile.T @ wo-slice  (partial output projection)
"""

import numpy as np

import bass_rust
import concourse.bass as bass
import concourse.mybir as mybir
import concourse.tile as tile
from concourse import bass_utils

F32 = mybir.dt.float32
F32R = mybir.dt.float32r

B, T, C = 2, 2048, 2048
H, KVH, HD = 16, 4, 128
HL = H // KVH          # q-heads per core (local)
P = 128                # partitions / tile edge
TB = 512               # t/i block (moving free dim)
NTB = T // TB          # 4
NC = C // P            # 16 contraction tiles
SCALE = float(HD) ** -0.25  # folded into both q and k norms
PIPELINE = False


def split_multi_waits(nc):
    """This container's walrus accepts at most ONE sync-wait per instruction;
    Tile can attach several. Move extras onto single-wait NoOps inserted just
    before the instruction on the same engine."""
    seq = 0
    for f in nc.m.functions:
        for bb in f.blocks:
            new = []
            changed = False
            for inst in bb.instructions:
                si = inst.sync_info
                waits = list(si.on_wait) if (si is not None and si.on_wait) else []
                if len(waits) > 1 and type(inst).__name__ != "DMACopy":
                    for w in waits[:-1]:
                        nop = bass_rust.InstNoOp(name=f"I-waitsplit-{seq}")
                        seq += 1
                        nop.engine = inst.engine
                        nop.sync_info = mybir.SyncInfo(on_wait=[w], on_update=[])
                        nc.register_instruction(nop, overwrite=True)
                        new.append(nop)
                    si.on_wait = waits[-1:]
                    changed = True
                new.append(inst)
            if changed:
                bb.instructions = new


def build_nc():
    nc = bass.Bass()

    xt = nc.declare_dram_parameter("xt", [C, T], F32R, isOutput=False)
    wq = nc.declare_dram_parameter("wq", [C, HL * HD], F32R, isOutput=False)
    wk = nc.declare_dram_parameter("wk", [C, HD], F32R, isOutput=False)
    wv = nc.declare_dram_parameter("wv", [C, HD], F32R, isOutput=False)
    wo = nc.declare_dram_parameter("wo", [HL * HD, C], F32R, isOutput=False)
    cc = nc.declare_dram_parameter("cc", [P, T], F32, isOutput=False)
    ssn = nc.declare_dram_parameter("ssn", [P, T], F32, isOutput=False)
    masks = nc.declare_dram_parameter("masks", [P, HL * TB], F32R, isOutput=False)
    qkw_row = nc.declare_dram_parameter("qkw_row", [1, P], F32R, isOutput=False)
    cols = nc.declare_dram_parameter("cols", [P, 2], F32R, isOutput=False)
    onr = nc.declare_dram_parameter("onr", [1, P], F32R, isOutput=False)
    ident = nc.declare_dram_parameter("ident", [P, P], F32R, isOutput=False)
    out = nc.declare_dram_parameter("out", [T, C], F32, isOutput=True)

    Exp = mybir.ActivationFunctionType.Exp
    Ln = mybir.ActivationFunctionType.Ln
    Copy = mybir.ActivationFunctionType.Copy

    with tile.TileContext(nc) as tc:
        with (
            tc.tile_pool(name="const", bufs=1) as const,
            tc.tile_pool(name="wpool", bufs=1) as wpool,
            tc.tile_pool(name="kv", bufs=1) as kvpool,
            tc.tile_pool(name="qt", bufs=2) as qtpool,
            tc.tile_pool(name="ot", bufs=2) as otpool,
            tc.tile_pool(name="xtp", bufs=3) as xtp,
            tc.tile_pool(name="work", bufs=2) as work,
            tc.tile_pool(name="pexp", bufs=3) as pexpp,
            tc.tile_pool(name="osb", bufs=2) as osb,
            tc.tile_pool(name="small", bufs=1) as small,
            tc.tile_pool(name="ps", bufs=1, space="PSUM") as ps,
        ):
            # ---- resident constants / weights ----
            masks_sb = const.tile([P, HL, TB], F32R, tag="masks")
            nc.sync.dma_start(out=masks_sb, in_=masks[:, :].rearrange("p (r i) -> p r i", r=HL))
            qkw_sb = const.tile([1, P], F32R, tag="qkw")
            nc.sync.dma_start(out=qkw_sb, in_=qkw_row[:, :])
            cols_sb = const.tile([P, 2], F32R, tag="cols")
            nc.sync.dma_start(out=cols_sb, in_=cols[:, :])
            onr_sb = const.tile([1, P], F32R, tag="onr")
            nc.sync.dma_start(out=onr_sb, in_=onr[:, :])
            id_sb = const.tile([P, P], F32R, tag="ident")
            nc.sync.dma_start(out=id_sb, in_=ident[:, :])
            eps_sb = const.tile([1, 1], F32, tag="eps")
            nc.vector.memset(eps_sb, 1e-6)
            cc_sb = const.tile([P, T], F32, tag="cc")
            nc.sync.dma_start(out=cc_sb, in_=cc[:, :])
            ssn_sb = const.tile([P, T], F32, tag="ssn")
            nc.sync.dma_start(out=ssn_sb, in_=ssn[:, :])

            wq_sb = wpool.tile([P, NC, HL * HD], F32R, tag="wq")
            nc.sync.dma_start(out=wq_sb, in_=wq[:, :].rearrange("(c p) f -> p c f", p=P))
            wk_sb = wpool.tile([P, NC, HD], F32R, tag="wk")
            nc.sync.dma_start(out=wk_sb, in_=wk[:, :].rearrange("(c p) f -> p c f", p=P))
            wv_sb = wpool.tile([P, NC, HD], F32R, tag="wv")
            nc.sync.dma_start(out=wv_sb, in_=wv[:, :].rearrange("(c p) f -> p c f", p=P))
            wo_sb = wpool.tile([P, HL, C], F32R, tag="wo")
            nc.sync.dma_start(out=wo_sb, in_=wo[:, :].rearrange("(h p) f -> p h f", p=P))

            # K^T and V-natural stay resident across the whole kernel
            kt_sb = kvpool.tile([P, T], F32R, tag="kt")
            vn_sb = kvpool.tile([P, NC, HD], F32R, tag="vn")

            def rope_rms_epilogue(src_ps, dst, cc_t, ssn_t):
                """src_ps: [128(d), 512(t)] PSUM projection block (pre-norm Q^T/K^T).
                dst: [128, 512] F32R SBUF destination slice."""
                # PSUM -> SBUF once on ACT so the DVE ops below run in 2x mode
                s_sb = work.tile([P, TB], F32, tag="src")
                nc.scalar.activation(out=s_sb, in_=src_ps, func=Copy)
                t1 = work.tile([P, TB], F32, tag="t1")
                nc.vector.tensor_mul(t1, s_sb, cc_t)
                t2 = work.tile([P, TB], F32, tag="t2")
                # ssn is host-arranged as [+sin; -sin] so each half-mul's two
                # SBUF inputs share a base partition (HW constraint)
                nc.vector.tensor_mul(t2[0:64, :], s_sb[64:128, :], ssn_t[64:128, :])
                nc.vector.tensor_mul(t2[64:128, :], s_sb[0:64, :], ssn_t[0:64, :])
                rot = t1
                nc.vector.tensor_add(rot, t1, t2)
                sq = work.tile([P, TB], F32R, tag="t2")
                nc.vector.tensor_mul(sq, rot, rot)
                var_ps = ps.tile([1, TB], F32, tag="aux", name="var")
                nc.tensor.matmul(var_ps, cols_sb[:, 1:2], sq, start=True, stop=True)
                # 1/sqrt(var+eps) = exp(-0.5 * ln(var+eps)); Ln/Exp share the
                # ACT function table with Exp/Copy so no table switches
                lnv = small.tile([1, TB], F32, tag="lnv")
                nc.scalar.activation(out=lnv, in_=var_ps, func=Ln, bias=eps_sb, scale=1.0)
                rstd_inv = small.tile([1, TB], F32R, tag="rstdi")
                nc.scalar.activation(out=rstd_inv, in_=lnv, func=Exp, scale=-0.5)
                bc_ps = ps.tile([P, TB], F32, tag="aux", name="bc")
                nc.tensor.matmul(bc_ps, qkw_sb, rstd_inv, start=True, stop=True)
                nc.vector.tensor_mul(dst, rot, bc_ps)

            qt_tiles = {}
            ot_tiles = {}

            def emit_phase1(tb):
                tsl = bass.ts(tb, TB)
                cc_t = cc_sb[:, tsl]
                ssn_t = ssn_sb[:, tsl]
                qt_sb = qtpool.tile([P, HL, TB], F32R, tag="qt", name="qt")
                qt_tiles[tb] = qt_sb
                rounds = [
                    [("q", 0), ("q", 1)],
                    [("q", 2), ("q", 3)],
                    [("k", 0), ("v", 0)],
                ]
                for rnd in rounds:
                    pts = {}
                    for idx, (kind, h) in enumerate(rnd):
                        pts[(kind, h)] = ps.tile([P, TB], F32, tag=f"proj{idx}", name=f"proj_{kind}{h}")
                    for c2 in range(NC // 2):
                        # two c-tiles per DMA, issued from the idle GPSIMD queue
                        # to keep the Sync engine off the critical path
                        xt_t = xtp.tile([P, 2, TB], F32R, tag="xt", name="xt")
                        nc.gpsimd.dma_start(
                            out=xt_t,
                            in_=xt[c2 * 2 * P:(c2 + 1) * 2 * P, tsl].rearrange(
                                "(c p) t -> p c t", p=P
                            ),
                        )
                        for ci in range(2):
                            c = 2 * c2 + ci
                            for kind, h in rnd:
                                if kind == "q":
                                    lhsT = wq_sb[:, c, h * HD:(h + 1) * HD]
                                elif kind == "k":
                                    lhsT = wk_sb[:, c, :]
                                else:
                                    lhsT = wv_sb[:, c, :]
                                nc.tensor.matmul(
                                    pts[(kind, h)], lhsT, xt_t[:, ci, :],
                                    start=(c == 0), stop=(c == NC - 1),
                                )
                    for kind, h in rnd:
                        src = pts[(kind, h)]
                        if kind == "q":
                            rope_rms_epilogue(src, qt_sb[:, h, :], cc_t, ssn_t)
                        elif kind == "k":
                            rope_rms_epilogue(src, kt_sb[:, tsl], cc_t, ssn_t)
                        else:
                            vt_sb = work.tile([P, TB], F32R, tag="vt", name="vt")
                            nc.vector.tensor_copy(out=vt_sb, in_=src)
                            for k in range(TB // P):
                                tr_ps = ps.tile([P, P], F32R, tag="aux", name="vtr")
                                nc.tensor.transpose(tr_ps, vt_sb[:, k * P:(k + 1) * P], id_sb)
                                nc.vector.tensor_copy(out=vn_sb[:, NTB * tb + k, :], in_=tr_ps)

            def emit_phase2(tb):
                nj = (HL * tb) + HL  # causal: j-tiles 0 .. 4*tb+3
                qt_sb = qt_tiles[tb]
                ot_sb = otpool.tile([P, HL, TB], F32R, tag="ot", name="ot")
                ot_tiles[tb] = ot_sb
                for h in range(HL):
                    o_ps = ps.tile([P, TB], F32, tag="oacc", name="oacc", bufs=1)
                    l_ps = ps.tile([1, TB], F32, tag="lps", name="lsum")
                    for j in range(nj):
                        sc_ps = ps.tile([P, TB], F32, tag="sc", name="sc", bufs=2)
                        nc.tensor.matmul(
                            sc_ps, kt_sb[:, j * P:(j + 1) * P], qt_sb[:, h, :],
                            start=True, stop=True,
                        )
                        pexp = pexpp.tile([P, TB], F32R, tag="pexp", name="pexp", bufs=4)
                        nc.scalar.activation(out=pexp, in_=sc_ps, func=Exp)
                        r = j - HL * tb
                        if r >= 0:
                            nc.vector.tensor_mul(pexp, pexp, masks_sb[:, r, :])
                        nc.tensor.matmul(
                            l_ps, cols_sb[:, 0:1], pexp,
                            start=(j == 0), stop=(j == nj - 1),
                        )
                        nc.tensor.matmul(
                            o_ps, vn_sb[:, j, :], pexp,
                            start=(j == 0), stop=(j == nj - 1),
                        )
                    lnl = small.tile([1, TB], F32, tag="lnl", name="lnl")
                    nc.scalar.activation(out=lnl, in_=l_ps, func=Ln)
                    linv = small.tile([1, TB], F32R, tag="linv", name="linv")
                    nc.scalar.activation(out=linv, in_=lnl, func=Exp, scale=-1.0)
                    bc2_ps = ps.tile([P, TB], F32, tag="aux", name="lbc")
                    nc.tensor.matmul(bc2_ps, onr_sb, linv, start=True, stop=True)
                    bc2_sb = work.tile([P, TB], F32, tag="bc2", bufs=1, name="bc2")
                    nc.vector.tensor_copy(out=bc2_sb, in_=bc2_ps)
                    nc.vector.tensor_mul(ot_sb[:, h, :], o_ps, bc2_sb)

            def emit_phase3(tb):
                ot_sb = ot_tiles[tb]
                for ts_i in range(TB // P):
                    trow = tb * TB + ts_i * P
                    for nb in range(C // TB):
                        out_ps = ps.tile([P, TB], F32, tag="outp", name="outp")
                        for h in range(HL):
                            nc.tensor.matmul(
                                out_ps,
                                ot_sb[:, h, ts_i * P:(ts_i + 1) * P],
                                wo_sb[:, h, nb * TB:(nb + 1) * TB],
                                start=(h == 0), stop=(h == HL - 1),
                            )
                        o_out = osb.tile([P, TB], F32, tag="osb", name="osb")
                        if (ts_i + nb) % 2 == 0:
                            nc.vector.tensor_copy(out=o_out, in_=out_ps)
                        else:
                            nc.scalar.activation(out=o_out, in_=out_ps, func=Copy)
                        nc.sync.dma_start(
                            out=out[trow:trow + P, nb * TB:(nb + 1) * TB], in_=o_out
                        )

            if PIPELINE:
                emit_phase1(0)
                emit_phase1(1)
                for tb in range(NTB):
                    emit_phase2(tb)
                    emit_phase3(tb)
                    if tb + 2 < NTB:
                        emit_phase1(tb + 2)
            else:
                for tb in range(NTB):
                    emit_phase1(tb)
                    emit_phase2(tb)
                    emit_phase3(tb)

    split_multi_waits(nc)
    return nc


_CACHED = {}


def _get_nc():
    if "nc" not in _CACHED:
        _CACHED["nc"] = build_nc()
    return _CACHED["nc"]


def _host_inputs(x, cos, sin, wq, wk, wv, wo, qk_w):
    """Build the 8 per-core input maps."""
    x = np.asarray(x, np.float32)
    cos = np.asarray(cos, np.float32)
    sin = np.asarray(sin, np.float32)
    wq = np.asarray(wq, np.float32)
    wk = np.asarray(wk, np.float32)
    wv = np.asarray(wv, np.float32)
    wo = np.asarray(wo, np.float32)
    qk_w = np.asarray(qk_w, np.float32)

    cc = np.ascontiguousarray(np.concatenate([cos.T, cos.T], axis=0))      # [128, T]
    # [ +sin ; -sin ]: row d<64 holds sin (pairs with q[d] into rot[d+64]),
    # row d>=64 holds -sin (pairs with q[d] into rot[d-64])
    ssn = np.ascontiguousarray(np.concatenate([sin.T, -sin.T], axis=0))    # [128, T]
    jl = np.arange(P)[:, None]
    il = np.arange(TB)[None, :]
    masks = np.concatenate(
        [(jl + P * r <= il).astype(np.float32) for r in range(HL)], axis=1
    )                                                                      # [128, 4*512]
    qkw_row = np.ascontiguousarray((qk_w * SCALE)[None, :])                # [1, 128]
    cols = np.stack(
        [np.ones(P, np.float32), np.full(P, 1.0 / HD, np.float32)], axis=1
    )                                                                      # [128, 2]
    onr = np.ones((1, P), np.float32)
    ident = np.eye(P, dtype=np.float32)

    ins = []
    for core in range(8):
        b, g = divmod(core, KVH)
        ins.append({
            "xt": np.ascontiguousarray(x[b].T),
            "wq": np.ascontiguousarray(wq[:, g * HL * HD:(g + 1) * HL * HD]),
            "wk": np.ascontiguousarray(wk[:, g * HD:(g + 1) * HD]),
            "wv": np.ascontiguousarray(wv[:, g * HD:(g + 1) * HD]),
            "wo": np.ascontiguousarray(wo[g * HL * HD:(g + 1) * HL * HD, :]),
            "cc": cc, "ssn": ssn, "masks": masks, "qkw_row": qkw_row,
            "cols": cols, "onr": onr, "ident": ident,
        })
    return ins


def run_cores(ins, trace=False, **kwargs):
    nc = _get_nc()
    return bass_utils.run_bass_kernel_spmd(
        nc, ins, list(range(8)), trace=trace, **kwargs
    )


def kernel(x, cos, sin, wq, wk, wv, wo, qk_w):
    ins = _host_inputs(x, cos, sin, wq, wk, wv, wo, qk_w)
    res = run_cores(ins, trace=False)
    out = np.zeros((B, T, C), np.float32)
    for core in range(8):
        b = core // KVH
        out[b] += res.results[core]["out"]
    return out
